# revision 2
# baseline (speedup 1.0000x reference)
"""CostVolume kernel for Trainium2 (8 NeuronCores, batch-sharded).

out[b,h,w,(di,dj)] = mean_c( prv[b,h,w,c] * nxt_pad[b,h+di,w+dj,c] ),  r=4, d=9.

Device strategy (per core, 2 batches):
  - Inputs host-prepped: prv scaled by 1/C, both cast to bf16 and transposed
    to [b, c, h, w] so the channel dim lands on SBUF partitions.
  - TensorEngine: for each (b, h): stationary lhsT = prv row [c, w(128)],
    moving rhs = 9 shifted nxt rows [c, w'] -> PSUM band tile
    [w(128 part), (di 9, w' 128)] f32, contracting c in 2 chunks (128+64).
  - DVE + ACT split the PSUM->SBUF copy (bf16 out).
  - DMA band tiles to DRAM; host gathers the 9 diagonals per (w, di)
    (a skew is not expressible as a Trainium access pattern) into the
    final [B,H,W,81] f32 output during unshard.
"""

import numpy as np
import ml_dtypes

B, H, W, C = 16, 128, 128, 192
R = 4
D = 2 * R + 1  # 9
N_CORES = 8
B_LOC = B // N_CORES  # 2
C0 = 128  # first contraction chunk
C1 = C - C0  # 64
HB = 8  # h rows per DMA block
N_HB = H // HB  # 16

_CACHED = {}


def _build_nc():
    import concourse.mybir as mybir
    from concourse.bacc import Bacc
    from concourse.tile import TileContext

    fp32 = mybir.dt.float32
    bf16 = mybir.dt.bfloat16

    nc = Bacc(
        "TRN2",
        target_bir_lowering=False,
        debug=False,
        num_devices=N_CORES,
    )

    prv_d = nc.dram_tensor("prv_t", [B_LOC, C, H, W], bf16, kind="ExternalInput")
    nxt_d = nc.dram_tensor("nxt_t", [B_LOC, C, H, W], bf16, kind="ExternalInput")
    band_d = nc.dram_tensor("band", [B_LOC, H, W, D, W], bf16, kind="ExternalOutput")

    with TileContext(nc) as tc:
        with (
            tc.tile_pool(name="prv_pool", bufs=2) as prv_pool,
            tc.tile_pool(name="nxt_pool", bufs=4) as nxt_pool,
            tc.tile_pool(name="band_pool", bufs=3) as band_pool,
            tc.tile_pool(name="psum_pool", bufs=2, space="PSUM") as psum_pool,
        ):
            for b in range(B_LOC):
                nxt_tiles = {}  # hb -> (c0_tile, c1_tile)

                def load_nxt(hb):
                    t0 = nxt_pool.tile([C0, HB, W], bf16, tag="nxt_c0")
                    t1 = nxt_pool.tile([C1, HB, W], bf16, tag="nxt_c1")
                    nc.sync.dma_start(t0[:], nxt_d[b, 0:C0, hb * HB:(hb + 1) * HB, :])
                    nc.sync.dma_start(t1[:], nxt_d[b, C0:C, hb * HB:(hb + 1) * HB, :])
                    nxt_tiles[hb] = (t0, t1)

                load_nxt(0)
                load_nxt(1)

                for hb in range(N_HB):
                    if hb + 2 < N_HB and (hb + 2) not in nxt_tiles:
                        load_nxt(hb + 2)

                    p0 = prv_pool.tile([C0, HB, W], bf16, tag="prv_c0")
                    p1 = prv_pool.tile([C1, HB, W], bf16, tag="prv_c1")
                    nc.sync.dma_start(p0[:], prv_d[b, 0:C0, hb * HB:(hb + 1) * HB, :])
                    nc.sync.dma_start(p1[:], prv_d[b, C0:C, hb * HB:(hb + 1) * HB, :])

                    for hl in range(HB):
                        h = hb * HB + hl
                        di_lo = max(0, R - h)
                        di_hi = min(D, H + R - h)

                        psum = psum_pool.tile([W, D, W], fp32, tag="band_ps")
                        for di in range(di_lo, di_hi):
                            h2 = h + di - R
                            for chunk in range(2):
                                lhsT = (p0 if chunk == 0 else p1)[:, hl, :]
                                t = nxt_tiles[h2 // HB][chunk]
                                rhs = t[:, h2 % HB, :]
                                nc.tensor.matmul(
                                    psum[:, di, :], lhsT, rhs,
                                    start=(chunk == 0), stop=(chunk == 1),
                                )

                        band = band_pool.tile([W, D, W], bf16, tag="band_sb")
                        # split the PSUM->SBUF copy between DVE and ACT
                        n_val = di_hi - di_lo
                        di_mid = di_lo + (n_val + 1) // 2
                        nc.vector.tensor_copy(
                            band[:, di_lo:di_mid, :], psum[:, di_lo:di_mid, :]
                        )
                        if di_mid < di_hi:
                            nc.scalar.copy(
                                band[:, di_mid:di_hi, :], psum[:, di_mid:di_hi, :]
                            )
                        nc.sync.dma_start(
                            band_d[b, h, :, di_lo:di_hi, :], band[:, di_lo:di_hi, :]
                        )

    nc.finalize()
    return nc


def _get_nc():
    if "nc" not in _CACHED:
        _CACHED["nc"] = _build_nc()
    return _CACHED["nc"]


def _host_prep(prv, nxt):
    """Scale prv by 1/C, cast to bf16, transpose to [b, c, h, w]."""
    bf16 = ml_dtypes.bfloat16
    prv_t = np.ascontiguousarray(
        (np.asarray(prv, dtype=np.float32) * (1.0 / C)).transpose(0, 3, 1, 2)
    ).astype(bf16)
    nxt_t = np.ascontiguousarray(
        np.asarray(nxt, dtype=np.float32).transpose(0, 3, 1, 2)
    ).astype(bf16)
    return prv_t, nxt_t


def _gather_band(band):
    """band: [B_LOC, H, W, D, W'] (bf16) -> out [B_LOC, H, W, D*D] f32.

    out[b,h,w,di,dj] = band[b,h,w,di, w+dj-R] where the h/w displaced indices
    are in range, else 0.
    """
    band = np.asarray(band, dtype=np.float32)
    padded = np.zeros((B_LOC, H, W, D, W + 2 * R), dtype=np.float32)
    padded[..., R:R + W] = band
    # gather along last axis: idx[w, dj] = w + dj
    idx = (np.arange(W)[:, None] + np.arange(D)[None, :])  # [W, D]
    idx = idx[None, None, :, None, :]  # [1,1,W,1,D]
    idx = np.broadcast_to(idx, (B_LOC, H, W, D, D))
    out = np.take_along_axis(padded, idx, axis=-1)  # [B_LOC, H, W, D(di), D(dj)]
    # zero rows where h + di - R out of range (those band slices are never
    # written on device -> may hold garbage)
    h_idx = np.arange(H)[:, None] + np.arange(D)[None, :] - R  # [H, D]
    h_valid = (h_idx >= 0) & (h_idx < H)  # [H, D]
    out = np.where(h_valid[None, :, None, :, None], out, np.float32(0.0))
    return out.reshape(B_LOC, H, W, D * D)


def _make_in_maps(prv, nxt):
    prv_t, nxt_t = _host_prep(prv, nxt)
    return [
        {
            "prv_t": prv_t[i * B_LOC:(i + 1) * B_LOC],
            "nxt_t": nxt_t[i * B_LOC:(i + 1) * B_LOC],
        }
        for i in range(N_CORES)
    ]


def kernel(prv, nxt, search_range):
    from concourse.bass_utils import run_bass_kernel_spmd

    assert int(search_range) == R
    prv = np.asarray(prv)
    nxt = np.asarray(nxt)
    assert prv.shape == (B, H, W, C), prv.shape
    out_dtype = prv.dtype if prv.dtype in (np.float32,) else np.float32

    in_maps = _make_in_maps(prv, nxt)

    nc = _get_nc()
    res = run_bass_kernel_spmd(nc, in_maps, list(range(N_CORES)))

    out = np.empty((B, H, W, D * D), dtype=out_dtype)
    for i in range(N_CORES):
        out[i * B_LOC:(i + 1) * B_LOC] = _gather_band(res.results[i]["band"])
    return out



# revision 7
# speedup vs baseline: 3.6841x; 3.6841x over previous
"""CostVolume kernel for Trainium2 (8 NeuronCores, batch-sharded).

out[b,h,w,(di,dj)] = mean_c( prv[b,h,w,c] * nxt_pad[b,h+di,w+dj,c] ),  r=4, d=9.

Device strategy (per core, 2 batches):
  - Host prep: prv scaled by 1/C -> bf16, transposed to [b, c, h, w];
    nxt -> bf16, transposed and zero-padded to [b, c, 136, 136].
  - Each matmul covers a 16x8-pixel patch of prv (M = 128 PSUM partitions
    = pixels) against its 24x16-pixel nxt window (N = 384), contracting
    c in 2 chunks (128 + 64). 2 b x 8 I x 16 J = 256 patches/core.
  - DVE/ACT alternate the PSUM->SBUF bf16 copy; band tiles batch 16
    J-patches -> 16 DMAs of 1.57 MB per core.
  - Host gathers out[b,16I+i,8J+j,di,dj] = band[b,I,p=i*8+j,J,(i+di)*16+j+dj]
    (the per-pixel diagonal is not a Trainium access pattern).
"""

import numpy as np
import ml_dtypes

B, H, W, C = 16, 128, 128, 192
R = 4
D = 2 * R + 1  # 9
N_CORES = 8
B_LOC = B // N_CORES  # 2
C0 = 128  # first contraction chunk
C1 = C - C0  # 64
PH, PW = 16, 8  # patch size (h, w); PH*PW = 128 = M
WH, WW = PH + 2 * R, PW + 2 * R  # 24, 16 window
NB = WH * WW  # 384 band columns per patch
NI = H // PH  # 8 patch rows
NJ = W // PW  # 16 patch cols
HP = H + 2 * R  # 136 padded

_CACHED = {}


def _build_nc():
    import concourse.mybir as mybir
    from concourse.bacc import Bacc
    from concourse.tile import TileContext

    fp32 = mybir.dt.float32
    bf16 = mybir.dt.bfloat16

    nc = Bacc(
        "TRN2",
        target_bir_lowering=False,
        debug=False,
        num_devices=N_CORES,
    )

    prv_d = nc.dram_tensor(
        "prv_t", [B_LOC, C, NI, NJ, PH * PW], bf16, kind="ExternalInput"
    )
    nxt_d = nc.dram_tensor("nxt_p", [B_LOC, C, HP, HP], bf16, kind="ExternalInput")
    band_d = nc.dram_tensor(
        "band", [B_LOC, NI, PH * PW, NJ, NB], bf16, kind="ExternalOutput"
    )

    with TileContext(nc) as tc:
        with (
            tc.tile_pool(name="nxt0_pool", bufs=2) as nxt0_pool,
            tc.tile_pool(name="nxt1_pool", bufs=2) as nxt1_pool,
            tc.tile_pool(name="prv_pool", bufs=3) as prv_pool,
            tc.tile_pool(name="band_pool", bufs=2) as band_pool,
            tc.tile_pool(name="psum_pool", bufs=8, space="PSUM") as psum_pool,
        ):
            for b in range(B_LOC):
                n0 = nxt0_pool.tile([C0, HP, HP], bf16, tag="nxt_c0")
                n1 = nxt1_pool.tile([C1, HP, HP], bf16, tag="nxt_c1")
                nc.sync.dma_start(n0[:], nxt_d[b, 0:C0])
                nc.sync.dma_start(n1[:], nxt_d[b, C0:C])

                prv_tiles = {}

                def load_prv(i):
                    p0 = prv_pool.tile([C0, NJ, PH * PW], bf16, tag="prv_c0")
                    p1 = prv_pool.tile([C1, NJ, PH * PW], bf16, tag="prv_c1")
                    nc.sync.dma_start(p0[:], prv_d[b, 0:C0, i])
                    nc.sync.dma_start(p1[:], prv_d[b, C0:C, i])
                    prv_tiles[i] = (p0, p1)

                load_prv(0)
                load_prv(1)

                for i in range(NI):
                    if i + 2 < NI:
                        load_prv(i + 2)
                    p0, p1 = prv_tiles.pop(i)
                    band = band_pool.tile([PH * PW, NJ, NB], bf16, tag="band_sb")
                    for j in range(NJ):
                        psum = psum_pool.tile([PH * PW, NB], fp32, tag="band_ps")
                        nc.tensor.matmul(
                            psum[:],
                            p0[:, j, :],
                            n0[:, i * PH:i * PH + WH, j * PW:j * PW + WW],
                            start=True,
                            stop=False,
                        )
                        nc.tensor.matmul(
                            psum[:],
                            p1[:, j, :],
                            n1[:, i * PH:i * PH + WH, j * PW:j * PW + WW],
                            start=False,
                            stop=True,
                        )
                        if j % 2 == 0:
                            nc.vector.tensor_copy(band[:, j, :], psum[:])
                        else:
                            nc.scalar.copy(band[:, j, :], psum[:])
                    nc.sync.dma_start(band_d[b, i], band[:])

    nc.finalize()
    return nc


def _get_nc():
    if "nc" not in _CACHED:
        _CACHED["nc"] = _build_nc()
    return _CACHED["nc"]


def _host_prep(prv, nxt):
    """prv: scale by 1/C, bf16, patch-major [b, c, I, J, 128].
    nxt: bf16, [b, c, h, w] zero-padded to 136x136."""
    bf16 = ml_dtypes.bfloat16
    prv_t = (np.asarray(prv, dtype=np.float32) * (1.0 / C)).transpose(0, 3, 1, 2)
    prv_t = prv_t.reshape(B, C, NI, PH, NJ, PW).transpose(0, 1, 2, 4, 3, 5)
    prv_t = np.ascontiguousarray(prv_t.reshape(B, C, NI, NJ, PH * PW)).astype(bf16)
    nxt_t = np.asarray(nxt, dtype=np.float32).transpose(0, 3, 1, 2).astype(bf16)
    nxt_p = np.zeros((B, C, HP, HP), dtype=bf16)
    nxt_p[:, :, R:R + H, R:R + W] = nxt_t
    return prv_t, nxt_p


def _make_in_maps(prv, nxt):
    prv_t, nxt_p = _host_prep(prv, nxt)
    return [
        {
            "prv_t": prv_t[i * B_LOC:(i + 1) * B_LOC],
            "nxt_p": nxt_p[i * B_LOC:(i + 1) * B_LOC],
        }
        for i in range(N_CORES)
    ]


# gather index: n[p=(i,j), di, dj] = (i+di)*WW + (j+dj)
_ii, _jj = np.meshgrid(np.arange(PH), np.arange(PW), indexing="ij")
_di, _dj = np.meshgrid(np.arange(D), np.arange(D), indexing="ij")
_GIDX = (
    (_ii.reshape(-1)[:, None, None] + _di[None]) * WW
    + (_jj.reshape(-1)[:, None, None] + _dj[None])
).reshape(1, 1, 1, PH * PW, D * D)  # [1,1,1,128,81]


def _gather_band(band):
    """band: [B_LOC, NI, 128, NJ, NB] bf16 -> out [B_LOC, H, W, D*D] f32."""
    band = np.asarray(band, dtype=np.float32)
    band = band.transpose(0, 1, 3, 2, 4)  # [b, I, J, p, NB]
    idx = np.broadcast_to(_GIDX, band.shape[:3] + (PH * PW, D * D))
    out = np.take_along_axis(band, idx, axis=-1)  # [b, I, J, 128, 81]
    out = out.reshape(B_LOC, NI, NJ, PH, PW, D * D)
    out = out.transpose(0, 1, 3, 2, 4, 5)  # [b, I, i, J, j, 81]
    return np.ascontiguousarray(out.reshape(B_LOC, H, W, D * D))


def kernel(prv, nxt, search_range):
    from concourse.bass_utils import run_bass_kernel_spmd

    assert int(search_range) == R
    prv = np.asarray(prv)
    nxt = np.asarray(nxt)
    assert prv.shape == (B, H, W, C), prv.shape

    in_maps = _make_in_maps(prv, nxt)

    nc = _get_nc()
    res = run_bass_kernel_spmd(nc, in_maps, list(range(N_CORES)))

    out = np.empty((B, H, W, D * D), dtype=np.float32)
    for i in range(N_CORES):
        out[i * B_LOC:(i + 1) * B_LOC] = _gather_band(res.results[i]["band"])
    return out


# revision 8
# speedup vs baseline: 3.7918x; 1.0292x over previous
"""CostVolume kernel for Trainium2 (8 NeuronCores, batch-sharded).

out[b,h,w,(di,dj)] = mean_c( prv[b,h,w,c] * nxt_pad[b,h+di,w+dj,c] ),  r=4, d=9.

Device strategy (per core, 2 batches):
  - Host prep: prv scaled by 1/C -> bf16, patch-major [b, c', I, J, 128];
    nxt -> bf16, [b, c', 136, 136] zero-padded. c' = 256 = [c0..127,
    c128..191, c128..191]: the 64-channel second contraction chunk is
    duplicated so it exists on SBUF partitions 0-63 AND 64-127.
  - Per 16x8-pixel patch: matmul (M=128 pixels, N=384 = 24x16 nxt window)
    contracting c. Chunk1 (K=128) per patch; chunk2 (K=64) of two adjacent
    patches runs CONCURRENTLY on PE row-groups 0-1 / 2-3 via tile_position.
  - DVE/ACT alternate the PSUM->SBUF bf16 copy; per (b, I) the band tile
    batches 16 J-patches, then 16 row-group-window DMAs emit only
    band[8g:8g+8, :, 16g:16g+144] (the only columns group g ever needs).
  - Host gathers out[...] = band_g[b,I,i,j,J, 16*di + j + dj].
"""

import numpy as np
import ml_dtypes

B, H, W, C = 16, 128, 128, 192
R = 4
D = 2 * R + 1  # 9
N_CORES = 8
B_LOC = B // N_CORES  # 2
C0 = 128  # first contraction chunk
C1 = C - C0  # 64
CD = C0 + 2 * C1  # 256: chunk1 + chunk2 duplicated
PH, PW = 16, 8  # patch size (h, w); PH*PW = 128 = M
WH, WW = PH + 2 * R, PW + 2 * R  # 24, 16 window
NB = WH * WW  # 384 band columns per patch
GW = 144  # per-row-group band window width
NI = H // PH  # 8 patch rows
NJ = W // PW  # 16 patch cols
HP = H + 2 * R  # 136 padded
NSL = 4  # h-slices per nxt load

_CACHED = {}


def _build_nc():
    import concourse.mybir as mybir
    from concourse.bacc import Bacc
    from concourse.tile import TileContext

    fp32 = mybir.dt.float32
    bf16 = mybir.dt.bfloat16

    nc = Bacc(
        "TRN2",
        target_bir_lowering=False,
        debug=False,
        num_devices=N_CORES,
    )

    prv_d = nc.dram_tensor(
        "prv_t", [B_LOC, CD, NI, NJ, PH * PW], bf16, kind="ExternalInput"
    )
    nxt_d = nc.dram_tensor("nxt_p", [B_LOC, CD, HP, HP], bf16, kind="ExternalInput")
    band_d = nc.dram_tensor(
        "band", [B_LOC, NI, PH, PW, NJ, GW], bf16, kind="ExternalOutput"
    )

    slices = [(HP * s // NSL, HP * (s + 1) // NSL) for s in range(NSL)]

    with TileContext(nc) as tc:
        with (
            tc.tile_pool(name="nxt0_pool", bufs=2) as nxt0_pool,
            tc.tile_pool(name="nxt1_pool", bufs=2) as nxt1_pool,
            tc.tile_pool(name="prv_pool", bufs=3) as prv_pool,
            tc.tile_pool(name="band_pool", bufs=2) as band_pool,
            tc.tile_pool(name="psum_pool", bufs=8, space="PSUM") as psum_pool,
        ):
            for b in range(B_LOC):
                n0 = nxt0_pool.tile([C0, HP, HP], bf16, tag="nxt_c0")
                n1 = nxt1_pool.tile([C0, HP, HP], bf16, tag="nxt_c1")
                for lo, hi in slices:
                    nc.sync.dma_start(n0[:, lo:hi, :], nxt_d[b, 0:C0, lo:hi, :])
                    nc.sync.dma_start(n1[:, lo:hi, :], nxt_d[b, C0:CD, lo:hi, :])

                prv_tiles = {}

                def load_prv(i):
                    p0 = prv_pool.tile([C0, NJ, PH * PW], bf16, tag="prv_c0")
                    p1 = prv_pool.tile([C0, NJ, PH * PW], bf16, tag="prv_c1")
                    nc.sync.dma_start(p0[:], prv_d[b, 0:C0, i])
                    nc.sync.dma_start(p1[:], prv_d[b, C0:CD, i])
                    prv_tiles[i] = (p0, p1)

                load_prv(0)
                load_prv(1)

                for i in range(NI):
                    if i + 2 < NI:
                        load_prv(i + 2)
                    p0, p1 = prv_tiles.pop(i)
                    band = band_pool.tile([PH * PW, NJ, NB], bf16, tag="band_sb")
                    r0 = slice(i * PH, i * PH + WH)
                    for t in range(NJ // 2):
                        ja, jb = 2 * t, 2 * t + 1
                        ca = slice(ja * PW, ja * PW + WW)
                        cb = slice(jb * PW, jb * PW + WW)
                        psa = psum_pool.tile([PH * PW, NB], fp32, tag="band_ps")
                        psb = psum_pool.tile([PH * PW, NB], fp32, tag="band_ps")
                        nc.tensor.matmul(
                            psa[:], p0[:, ja, :], n0[:, r0, ca],
                            start=True, stop=False,
                        )
                        nc.tensor.matmul(
                            psb[:], p0[:, jb, :], n0[:, r0, cb],
                            start=True, stop=False,
                        )
                        nc.tensor.matmul(
                            psa[:], p1[0:C1, ja, :], n1[0:C1, r0, ca],
                            start=False, stop=True, tile_position=(0, 0),
                        )
                        nc.tensor.matmul(
                            psb[:], p1[C1:C0, jb, :], n1[C1:C0, r0, cb],
                            start=False, stop=True, tile_position=(64, 0),
                        )
                        nc.vector.tensor_copy(band[:, ja, :], psa[:])
                        nc.scalar.copy(band[:, jb, :], psb[:])
                    for g in range(PH):
                        nc.sync.dma_start(
                            band_d[b, i, g],
                            band[g * PW:(g + 1) * PW, :, g * WW:g * WW + GW],
                        )

    nc.finalize()
    return nc


def _get_nc():
    if "nc" not in _CACHED:
        _CACHED["nc"] = _build_nc()
    return _CACHED["nc"]


def _host_prep(prv, nxt):
    """prv: scale by 1/C, bf16, patch-major [b, 256, I, J, 128] with the
    64-channel chunk2 duplicated. nxt: bf16 [b, 256, 136, 136] padded."""
    bf16 = ml_dtypes.bfloat16
    prv_t = (np.asarray(prv, dtype=np.float32) * (1.0 / C)).transpose(0, 3, 1, 2)
    prv_t = prv_t.reshape(B, C, NI, PH, NJ, PW).transpose(0, 1, 2, 4, 3, 5)
    prv_t = prv_t.reshape(B, C, NI, NJ, PH * PW).astype(bf16)
    prv_t = np.ascontiguousarray(
        np.concatenate([prv_t, prv_t[:, C0:C]], axis=1)
    )  # [B, 256, NI, NJ, 128]
    nxt_t = np.asarray(nxt, dtype=np.float32).transpose(0, 3, 1, 2).astype(bf16)
    nxt_p = np.zeros((B, CD, HP, HP), dtype=bf16)
    nxt_p[:, 0:C, R:R + H, R:R + W] = nxt_t
    nxt_p[:, C:CD, R:R + H, R:R + W] = nxt_t[:, C0:C]
    return prv_t, nxt_p


def _make_in_maps(prv, nxt):
    prv_t, nxt_p = _host_prep(prv, nxt)
    return [
        {
            "prv_t": prv_t[i * B_LOC:(i + 1) * B_LOC],
            "nxt_p": nxt_p[i * B_LOC:(i + 1) * B_LOC],
        }
        for i in range(N_CORES)
    ]


# gather index: m[j, di, dj] = 16*di + j + dj  (within the 144-wide group window)
_j, _di, _dj = np.meshgrid(np.arange(PW), np.arange(D), np.arange(D), indexing="ij")
_GIDX = (WW * _di + _j + _dj).reshape(1, 1, 1, PW, 1, D * D)  # [1,1,1,8,1,81]


def _gather_band(band):
    """band: [B_LOC, NI, PH, PW, NJ, GW] bf16 -> out [B_LOC, H, W, D*D] f32."""
    band = np.asarray(band, dtype=np.float32)
    idx = np.broadcast_to(_GIDX, band.shape[:5] + (D * D,))
    out = np.take_along_axis(band, idx, axis=-1)  # [b, I, i, j, J, 81]
    out = out.transpose(0, 1, 2, 4, 3, 5)  # [b, I, i, J, j, 81]
    return np.ascontiguousarray(out.reshape(B_LOC, H, W, D * D))


def kernel(prv, nxt, search_range):
    from concourse.bass_utils import run_bass_kernel_spmd

    assert int(search_range) == R
    prv = np.asarray(prv)
    nxt = np.asarray(nxt)
    assert prv.shape == (B, H, W, C), prv.shape

    in_maps = _make_in_maps(prv, nxt)

    nc = _get_nc()
    res = run_bass_kernel_spmd(nc, in_maps, list(range(N_CORES)))

    out = np.empty((B, H, W, D * D), dtype=np.float32)
    for i in range(N_CORES):
        out[i * B_LOC:(i + 1) * B_LOC] = _gather_band(res.results[i]["band"])
    return out
